# revision 1
# baseline (speedup 1.0000x reference)
"""Trainium2 Bass kernel for the ContextModule sparse-attention problem.

Contract: kernel(**inputs) takes FULL unsharded numpy inputs and returns the
FULL outputs (ctx_out [512,128] f32, attn [512,16,32,32] f32).

Sharding: data-parallel over the clip dim N=8 — one clip per NeuronCore.
Each core gets its clip's context plus (up to 128, zero-padded) ROIs that
belong to that clip; projection weights are replicated. Host does the
gather/pad/layout; device does projection, keys/vals 1x1x1-conv, fused
cross-attention softmax, and both outputs; host scatters rows back.
"""

import math
import sys

for _p in ("/opt/trn_rl_repo", "/root/.axon_site/_ro/trn_rl_repo"):
    if _p not in sys.path:
        sys.path.insert(0, _p)

import numpy as np
import ml_dtypes

# Problem shapes (hardcoded per contract)
N, C, T, H, W = 8, 512, 16, 32, 32
S = T * H * W            # 16384 spatial positions per clip
DIN, DK, DV = 2048, 128, 128
NBP = 128                # padded boxes per core per round
TS = 512                 # spatial tile (one PSUM bank of fp32)
NT = S // TS             # 32 tiles
CCH = C // 128           # 4 contraction chunks over channels
ICH = DIN // 128         # 16 contraction chunks over roi features
N_CORES = 8

_STATE = {}


def _build():
    import concourse.bacc as bacc
    import concourse.mybir as mybir
    import concourse.tile as tile
    import concourse.bass as bass
    from contextlib import ExitStack

    f32 = mybir.dt.float32
    bf16 = mybir.dt.bfloat16
    AF = mybir.ActivationFunctionType

    nc = bacc.Bacc("TRN2", target_bir_lowering=False, debug=False,
                   num_devices=N_CORES)

    ctx_d = nc.dram_tensor("ctx", [NT, CCH, 128, TS], bf16, kind="ExternalInput").ap()
    rois_d = nc.dram_tensor("roisT", [128, ICH, NBP], bf16, kind="ExternalInput").ap()
    wproj_d = nc.dram_tensor("wprojT", [128, ICH, DK], bf16, kind="ExternalInput").ap()
    wkeys_d = nc.dram_tensor("wkeysT", [128, CCH, DK], bf16, kind="ExternalInput").ap()
    wvals_d = nc.dram_tensor("wvalsT", [128, CCH, DV], bf16, kind="ExternalInput").ap()
    bproj_d = nc.dram_tensor("bproj", [128, 1], f32, kind="ExternalInput").ap()
    bkeys_d = nc.dram_tensor("bkeys", [128, 1], f32, kind="ExternalInput").ap()
    bvals_d = nc.dram_tensor("bvalsb", [128, CCH, DV], f32, kind="ExternalInput").ap()
    ident_d = nc.dram_tensor("ident", [128, 128], f32, kind="ExternalInput").ap()
    ctxout_d = nc.dram_tensor("ctx_out", [NBP, DV], f32, kind="ExternalOutput").ap()
    attn_d = nc.dram_tensor("attn", [NBP, S], f32, kind="ExternalOutput").ap()

    inv_sqrt_dk = 1.0 / math.sqrt(DK)

    with tile.TileContext(nc) as tc, ExitStack() as ctx:
        const = ctx.enter_context(tc.tile_pool(name="const", bufs=1))
        big = ctx.enter_context(tc.tile_pool(name="big", bufs=1))
        ctxp = ctx.enter_context(tc.tile_pool(name="ctxp", bufs=3))
        keysp = ctx.enter_context(tc.tile_pool(name="keysp", bufs=3))
        coefp = ctx.enter_context(tc.tile_pool(name="coefp", bufs=3))
        outp = ctx.enter_context(tc.tile_pool(name="outp", bufs=2))
        ps_k = ctx.enter_context(tc.tile_pool(name="ps_k", bufs=2, space="PSUM"))
        ps_v = ctx.enter_context(tc.tile_pool(name="ps_v", bufs=2, space="PSUM"))
        ps_qk = ctx.enter_context(tc.tile_pool(name="ps_qk", bufs=2, space="PSUM"))
        ps_t = ctx.enter_context(tc.tile_pool(name="ps_t", bufs=1, space="PSUM"))
        ps_acc = ctx.enter_context(tc.tile_pool(name="ps_acc", bufs=1, space="PSUM"))

        # ---- constants / persistent buffers ----
        rois_t = const.tile([128, ICH, NBP], bf16)
        wproj_t = const.tile([128, ICH, DK], bf16)
        wkeys_t = const.tile([128, CCH, DK], bf16)
        wvals_t = const.tile([128, CCH, DV], bf16)
        bproj_t = const.tile([128, 1], f32)
        bkeys_t = const.tile([128, 1], f32)
        bvals_t = const.tile([128, CCH, DV], f32)
        ident_t = const.tile([128, 128], f32)
        qT_t = const.tile([128, NBP], bf16)
        sums_t = const.tile([128, NT], f32)
        total_t = const.tile([128, 1], f32)
        inv_t = const.tile([128, 1], f32)
        inv255_t = const.tile([128, 1], f32)

        exp_f = big.tile([128, NT, TS], f32)         # unnormalized exp(qk) rows
        vals_sv = big.tile([128, NT, CCH, DV], bf16)  # vals in [s, v] chunk layout

        nc.sync.dma_start(rois_t[:], rois_d[:])
        nc.sync.dma_start(wproj_t[:], wproj_d[:])
        nc.sync.dma_start(wkeys_t[:], wkeys_d[:])
        nc.sync.dma_start(wvals_t[:], wvals_d[:])
        nc.sync.dma_start(bproj_t[:], bproj_d[:])
        nc.sync.dma_start(bkeys_t[:], bkeys_d[:])
        nc.sync.dma_start(bvals_t[:], bvals_d[:])
        nc.sync.dma_start(ident_t[:], ident_d[:])

        # ---- roi query projection: qT[k, b] = W_proj @ roisT + b_proj ----
        q_ps = ps_t.tile([128, NBP], f32, tag="tp")
        for ic in range(ICH):
            nc.tensor.matmul(q_ps[:], wproj_t[:, ic, :], rois_t[:, ic, :],
                             start=(ic == 0), stop=(ic == ICH - 1))
        nc.scalar.activation(qT_t[:], q_ps[:], AF.Identity, bias=bproj_t[:])

        # ---- phase 1: stream context; keys, vals, qk, exp ----
        for ts in range(NT):
            ctile = ctxp.tile([128, CCH, TS], bf16)
            for cc in range(CCH):
                nc.sync.dma_start(ctile[:, cc, :], ctx_d[ts, cc])

            # keys[k, s-tile] accumulated over channel chunks
            kps = ps_k.tile([128, TS], f32)
            for cc in range(CCH):
                nc.tensor.matmul(kps[:], wkeys_t[:, cc, :], ctile[:, cc, :],
                                 start=(cc == 0), stop=(cc == CCH - 1))
            ksb = keysp.tile([128, TS], bf16)
            nc.vector.tensor_scalar_add(ksb[:], kps[:], bkeys_t[:])

            # qk[b, s-tile]
            qkps = ps_qk.tile([128, TS], f32)
            nc.tensor.matmul(qkps[:], qT_t[:], ksb[:], start=True, stop=True)
            nc.scalar.activation(exp_f[:, ts, :], qkps[:], AF.Exp,
                                 scale=inv_sqrt_dk,
                                 accum_out=sums_t[:, ts:ts + 1])

            # vals[s, v] directly in s-partition layout
            vps = ps_v.tile([128, CCH, DV], f32)
            for j in range(CCH):
                for cc in range(CCH):
                    nc.tensor.matmul(vps[:, j, :],
                                     ctile[:, cc, bass.ts(j, 128)],
                                     wvals_t[:, cc, :],
                                     start=(cc == 0), stop=(cc == CCH - 1))
            nc.vector.tensor_add(vals_sv[:, ts, :, :], vps[:], bvals_t[:])

        # ---- softmax scalars ----
        nc.vector.reduce_sum(total_t[:], sums_t[:], axis=mybir.AxisListType.X)
        nc.vector.reciprocal(inv_t[:], total_t[:])
        nc.vector.tensor_scalar_mul(inv255_t[:], inv_t[:], 255.0)

        # ---- phase 3: ctx_out = (exp^T chunks) @ vals_sv, then scale ----
        acc_ps = ps_acc.tile([128, DV], f32)
        for ts in range(NT):
            tps = ps_t.tile([128, CCH, 128], f32, tag="tp")
            for j in range(CCH):
                nc.tensor.transpose(tps[:, j, :], exp_f[:, ts, bass.ts(j, 128)],
                                    ident_t[:])
            coefT = coefp.tile([128, CCH, NBP], bf16)
            nc.vector.tensor_copy(coefT[:], tps[:])
            for j in range(CCH):
                g = ts * CCH + j
                nc.tensor.matmul(acc_ps[:], coefT[:, j, :], vals_sv[:, ts, j, :],
                                 start=(g == 0), stop=(g == NT * CCH - 1))

        ctxout_sb = outp.tile([128, DV], f32)
        nc.vector.tensor_scalar_mul(ctxout_sb[:], acc_ps[:], inv_t[:])
        nc.sync.dma_start(ctxout_d[:], ctxout_sb[:])

        # ---- attn = exp * (255 / sum), written out in 8 column blocks ----
        for blk in range(8):
            sl = exp_f[:, blk * 4:(blk + 1) * 4, :]
            nc.vector.tensor_scalar_mul(sl, sl, inv255_t[:])
            nc.sync.dma_start(attn_d[:, blk * 2048:(blk + 1) * 2048], sl)

    nc.compile()
    return nc


def _get_nc():
    if "nc" not in _STATE:
        _STATE["nc"] = _build()
    return _STATE["nc"]


def _numpy_fallback(rois, context, batch_idx, W_proj, b_proj, W_keys, b_keys,
                    W_vals, b_vals):
    n, c, t, h, w = context.shape
    s = t * h * w
    dk = W_keys.shape[0]
    q = rois @ W_proj.T + b_proj
    ctx_flat = context.reshape(n, c, s)
    keys = np.einsum('kc,ncs->nks', W_keys, ctx_flat) + b_keys[None, :, None]
    vals = np.einsum('vc,ncs->nsv', W_vals, ctx_flat) + b_vals[None, None, :]
    qk = np.einsum('bk,bks->bs', q, keys[batch_idx]) / math.sqrt(dk)
    qk = qk - qk.max(axis=-1, keepdims=True)
    e = np.exp(qk)
    coeff = e / e.sum(axis=-1, keepdims=True)
    ctx_out = np.einsum('bs,bsv->bv', coeff, vals[batch_idx])
    attn = coeff.reshape(-1, t, h, w) * 255.0
    return ctx_out.astype(np.float32), attn.astype(np.float32)


def kernel(rois, context, batch_idx, W_proj, b_proj, W_keys, b_keys,
           W_vals, b_vals):
    rois = np.asarray(rois, np.float32)
    context = np.asarray(context, np.float32)
    batch_idx = np.asarray(batch_idx)
    W_proj = np.asarray(W_proj, np.float32)
    b_proj = np.asarray(b_proj, np.float32)
    W_keys = np.asarray(W_keys, np.float32)
    b_keys = np.asarray(b_keys, np.float32)
    W_vals = np.asarray(W_vals, np.float32)
    b_vals = np.asarray(b_vals, np.float32)

    nb = rois.shape[0]
    if (context.shape != (N, C, T, H, W) or rois.shape[1] != DIN
            or W_proj.shape != (DK, DIN) or W_keys.shape != (DK, C)
            or W_vals.shape != (DV, C)):
        return _numpy_fallback(rois, context, batch_idx, W_proj, b_proj,
                               W_keys, b_keys, W_vals, b_vals)

    from concourse.bass_utils import run_bass_kernel_spmd

    nc = _get_nc()
    bf = ml_dtypes.bfloat16

    ctx_flat = context.reshape(N, C, S)
    # [N, NT, CCH, 128, TS] bf16
    ctx_bf = np.ascontiguousarray(
        ctx_flat.reshape(N, CCH, 128, NT, TS).transpose(0, 3, 1, 2, 4)
    ).astype(bf)

    wproj_h = np.ascontiguousarray(
        W_proj.T.reshape(ICH, 128, DK).transpose(1, 0, 2)).astype(bf)
    wkeys_h = np.ascontiguousarray(
        W_keys.T.reshape(CCH, 128, DK).transpose(1, 0, 2)).astype(bf)
    wvals_h = np.ascontiguousarray(
        W_vals.T.reshape(CCH, 128, DV).transpose(1, 0, 2)).astype(bf)
    bproj_h = np.ascontiguousarray(b_proj.reshape(DK, 1))
    bkeys_h = np.ascontiguousarray(b_keys.reshape(DK, 1))
    bvals_h = np.ascontiguousarray(np.broadcast_to(b_vals, (128, CCH, DV)))
    ident_h = np.eye(128, dtype=np.float32)

    idxs = [np.flatnonzero(batch_idx == n) for n in range(N)]
    rounds = max(1, max((len(ix) + NBP - 1) // NBP for ix in idxs))

    ctx_out_full = np.zeros((nb, DV), np.float32)
    attn_full = np.zeros((nb, S), np.float32)

    for r in range(rounds):
        in_maps = []
        sels = []
        for n in range(N):
            ix = idxs[n][r * NBP:(r + 1) * NBP]
            sels.append(ix)
            rp = np.zeros((NBP, DIN), np.float32)
            if len(ix):
                rp[:len(ix)] = rois[ix]
            rois_h = np.ascontiguousarray(
                rp.T.reshape(ICH, 128, NBP).transpose(1, 0, 2)).astype(bf)
            in_maps.append({
                "ctx": ctx_bf[n], "roisT": rois_h, "wprojT": wproj_h,
                "wkeysT": wkeys_h, "wvalsT": wvals_h, "bproj": bproj_h,
                "bkeys": bkeys_h, "bvalsb": bvals_h, "ident": ident_h,
            })
        res = run_bass_kernel_spmd(nc, in_maps, list(range(N_CORES)))
        for n in range(N):
            ix = sels[n]
            if len(ix):
                ctx_out_full[ix] = res.results[n]["ctx_out"][:len(ix)]
                attn_full[ix] = res.results[n]["attn"][:len(ix)]

    return ctx_out_full, attn_full.reshape(nb, T, H, W)


# revision 3
# speedup vs baseline: 1.4423x; 1.4423x over previous
"""Trainium2 Bass kernel for the ContextModule sparse-attention problem.

Contract: kernel(**inputs) takes FULL unsharded numpy inputs and returns the
FULL outputs (ctx_out [512,128] f32, attn [512,16,32,32] f32).

Sharding: data-parallel over the clip dim N=8 — one clip per NeuronCore.
Each core gets its clip's context plus (up to 128, zero-padded) ROIs that
belong to that clip; projection weights are replicated. Host does the
gather/pad/layout; device does projection, the fused qk/softmax/attention
matmuls; host applies the final softmax normalization to the attention map
while scattering rows back.

Device-side math per core (b = padded boxes, s = 16384 positions):
  qT[k,b]   = W_proj @ roisT + b_proj          (16 chunk matmuls)
  qWT[c,b]  = W_keys^T-chunks @ qT             (fold keys: qk = qW @ ctx)
  qbk[b]    = (q @ b_keys) / sqrt(dk)          (per-row bias for exp)
  per 512-col tile of ctx:
    qk[b,:]   = sum_cc qWT_cc^T @ ctx_cc       (PSUM accumulate)
    e[b,:]    = exp(qk/sqrt(dk) + qbk)  -> bf16, row-sums accumulated
    vals[s,v] = sum_cc ctx_cc-chunk^T @ W_vals^T-chunk + b_vals
    eT chunks = PE-transpose(e)                 (bf16)
    acc[b,v] += eT_chunk^T @ vals_chunk         (128-matmul PSUM accumulation)
  ctx_out = acc / row_sum ;  exp + row sums DMA'd out, host scales attn.
"""

import math
import sys

for _p in ("/opt/trn_rl_repo", "/root/.axon_site/_ro/trn_rl_repo"):
    if _p not in sys.path:
        sys.path.insert(0, _p)

import numpy as np
import ml_dtypes

# Problem shapes (hardcoded per contract)
N, C, T, H, W = 8, 512, 16, 32, 32
S = T * H * W            # 16384 spatial positions per clip
DIN, DK, DV = 2048, 128, 128
NBP = 128                # padded boxes per core per round
TS = 512                 # spatial tile (one PSUM bank of fp32)
NT = S // TS             # 32 tiles
CCH = C // 128           # 4 contraction chunks over channels
ICH = DIN // 128         # 16 contraction chunks over roi features
N_CORES = 8

_STATE = {}


def _build():
    import concourse.bacc as bacc
    import concourse.mybir as mybir
    import concourse.tile as tile
    import concourse.bass as bass
    from contextlib import ExitStack

    f32 = mybir.dt.float32
    bf16 = mybir.dt.bfloat16
    AF = mybir.ActivationFunctionType

    nc = bacc.Bacc("TRN2", target_bir_lowering=False, debug=False,
                   num_devices=N_CORES)

    # ctx layout: [NT, 128(part), CCH*TS] — 4KB/partition contiguous per tile
    ctx_d = nc.dram_tensor("ctx", [NT, 128, CCH, TS], bf16, kind="ExternalInput").ap()
    rois_d = nc.dram_tensor("roisT", [128, ICH, NBP], bf16, kind="ExternalInput").ap()
    wproj_d = nc.dram_tensor("wprojT", [128, ICH, DK], bf16, kind="ExternalInput").ap()
    wkeys_d = nc.dram_tensor("wkeysK", [128, CCH, 128], bf16, kind="ExternalInput").ap()
    wvals_d = nc.dram_tensor("wvalsT", [128, CCH, DV], bf16, kind="ExternalInput").ap()
    bproj_d = nc.dram_tensor("bproj", [128, 1], f32, kind="ExternalInput").ap()
    bkeys_d = nc.dram_tensor("bkeys", [128, 1], bf16, kind="ExternalInput").ap()
    bvals_d = nc.dram_tensor("bvalsb", [128, CCH, DV], f32, kind="ExternalInput").ap()
    ident_d = nc.dram_tensor("ident", [128, 128], bf16, kind="ExternalInput").ap()
    ctxout_d = nc.dram_tensor("ctx_out", [NBP, DV], f32, kind="ExternalOutput").ap()
    expout_d = nc.dram_tensor("expout", [NBP, S], bf16, kind="ExternalOutput").ap()
    sums_d = nc.dram_tensor("sums", [NBP, 1], f32, kind="ExternalOutput").ap()

    inv_sqrt_dk = 1.0 / math.sqrt(DK)

    with tile.TileContext(nc) as tc, ExitStack() as ctx:
        const = ctx.enter_context(tc.tile_pool(name="const", bufs=1))
        big = ctx.enter_context(tc.tile_pool(name="big", bufs=1))
        ctxp = ctx.enter_context(tc.tile_pool(name="ctxp", bufs=3))
        coefp = ctx.enter_context(tc.tile_pool(name="coefp", bufs=3))
        outp = ctx.enter_context(tc.tile_pool(name="outp", bufs=2))
        ps_qk = ctx.enter_context(tc.tile_pool(name="ps_qk", bufs=2, space="PSUM"))
        ps_v = ctx.enter_context(tc.tile_pool(name="ps_v", bufs=2, space="PSUM"))
        ps_t = ctx.enter_context(tc.tile_pool(name="ps_t", bufs=2, space="PSUM"))
        ps_acc = ctx.enter_context(tc.tile_pool(name="ps_acc", bufs=1, space="PSUM"))

        # ---- constants / persistent buffers ----
        rois_t = const.tile([128, ICH, NBP], bf16)
        wproj_t = const.tile([128, ICH, DK], bf16)
        wkeys_t = const.tile([128, CCH, 128], bf16)
        wvals_t = const.tile([128, CCH, DV], bf16)
        bproj_t = const.tile([128, 1], f32)
        bkeys_t = const.tile([128, 1], bf16)
        bvals_t = const.tile([128, CCH, DV], f32)
        ident_t = const.tile([128, 128], bf16)
        qT_t = const.tile([128, NBP], bf16)
        qWT_t = const.tile([128, CCH, NBP], bf16)
        qbk_t = const.tile([128, 1], f32)
        sums_t = const.tile([128, NT], f32)
        total_t = const.tile([128, 1], f32)
        inv_t = const.tile([128, 1], f32)

        exp_b = big.tile([128, NT, TS], bf16)         # unnormalized exp(qk) rows
        vals_sv = big.tile([128, NT, CCH, DV], bf16)  # vals in [s, v] chunk layout

        nc.sync.dma_start(rois_t[:], rois_d[:])
        nc.sync.dma_start(wproj_t[:], wproj_d[:])
        nc.sync.dma_start(wkeys_t[:], wkeys_d[:])
        nc.sync.dma_start(wvals_t[:], wvals_d[:])
        nc.sync.dma_start(bproj_t[:], bproj_d[:])
        nc.sync.dma_start(bkeys_t[:], bkeys_d[:])
        nc.sync.dma_start(bvals_t[:], bvals_d[:])
        nc.sync.dma_start(ident_t[:], ident_d[:])

        # ---- qT[k, b] = W_proj @ roisT + b_proj ----
        q_ps = ps_t.tile([128, NBP], f32, tag="tp")
        for ic in range(ICH):
            nc.tensor.matmul(q_ps[:], wproj_t[:, ic, :], rois_t[:, ic, :],
                             start=(ic == 0), stop=(ic == ICH - 1))
        nc.scalar.activation(qT_t[:], q_ps[:], AF.Identity, bias=bproj_t[:])

        # ---- qWT[c, b] chunks = W_keys-chunk^T... lhsT=W_keys[k, c-chunk] ----
        qw_ps = ps_t.tile([128, CCH, NBP], f32, tag="tp")
        for cc in range(CCH):
            nc.tensor.matmul(qw_ps[:, cc, :], wkeys_t[:, cc, :], qT_t[:],
                             start=True, stop=True)
        nc.vector.tensor_copy(qWT_t[:], qw_ps[:])

        # ---- qbk[b] = (q . b_keys) / sqrt(dk) : per-row exp bias ----
        qb_ps = ps_t.tile([128, 1], f32, tag="tp")
        nc.tensor.matmul(qb_ps[:], qT_t[:], bkeys_t[:], start=True, stop=True)
        nc.scalar.mul(qbk_t[:], qb_ps[:], inv_sqrt_dk)

        # ---- main loop over spatial tiles ----
        acc_ps = ps_acc.tile([128, DV], f32)
        for ts in range(NT):
            ctile = ctxp.tile([128, CCH, TS], bf16)
            nc.sync.dma_start(ctile[:], ctx_d[ts])

            # qk[b, s-tile] accumulated over channel chunks
            qkps = ps_qk.tile([128, TS], f32)
            for cc in range(CCH):
                nc.tensor.matmul(qkps[:], qWT_t[:, cc, :], ctile[:, cc, :],
                                 start=(cc == 0), stop=(cc == CCH - 1))
            nc.scalar.activation(exp_b[:, ts, :], qkps[:], AF.Exp,
                                 scale=inv_sqrt_dk, bias=qbk_t[:],
                                 accum_out=sums_t[:, ts:ts + 1])

            # vals[s, v] directly in s-partition layout
            vps = ps_v.tile([128, CCH, DV], f32)
            for j in range(CCH):
                for cc in range(CCH):
                    nc.tensor.matmul(vps[:, j, :],
                                     ctile[:, cc, bass.ts(j, 128)],
                                     wvals_t[:, cc, :],
                                     start=(cc == 0), stop=(cc == CCH - 1))
            nc.vector.tensor_add(vals_sv[:, ts, :, :], vps[:], bvals_t[:])

            # transpose exp chunks and accumulate ctx_out
            tps = ps_t.tile([128, CCH, 128], bf16, tag="tp")
            for j in range(CCH):
                nc.tensor.transpose(tps[:, j, :], exp_b[:, ts, bass.ts(j, 128)],
                                    ident_t[:])
            coefT = coefp.tile([128, CCH, NBP], bf16)
            nc.scalar.copy(coefT[:], tps[:])
            for j in range(CCH):
                g = ts * CCH + j
                nc.tensor.matmul(acc_ps[:], coefT[:, j, :], vals_sv[:, ts, j, :],
                                 start=(g == 0), stop=(g == NT * CCH - 1))

            # stream unnormalized exp rows out every 4 tiles (bf16)
            if ts % 4 == 3:
                blk = ts // 4
                nc.sync.dma_start(expout_d[:, blk * 2048:(blk + 1) * 2048],
                                  exp_b[:, blk * 4:(blk + 1) * 4, :])

        # ---- softmax scalars + ctx_out ----
        nc.vector.reduce_sum(total_t[:], sums_t[:], axis=mybir.AxisListType.X)
        nc.vector.reciprocal(inv_t[:], total_t[:])
        nc.sync.dma_start(sums_d[:], total_t[:])

        ctxout_sb = outp.tile([128, DV], f32)
        nc.vector.tensor_scalar_mul(ctxout_sb[:], acc_ps[:], inv_t[:])
        nc.sync.dma_start(ctxout_d[:], ctxout_sb[:])

    nc.compile()
    return nc


def _get_nc():
    if "nc" not in _STATE:
        _STATE["nc"] = _build()
    return _STATE["nc"]


def _numpy_fallback(rois, context, batch_idx, W_proj, b_proj, W_keys, b_keys,
                    W_vals, b_vals):
    n, c, t, h, w = context.shape
    s = t * h * w
    dk = W_keys.shape[0]
    q = rois @ W_proj.T + b_proj
    ctx_flat = context.reshape(n, c, s)
    keys = np.einsum('kc,ncs->nks', W_keys, ctx_flat) + b_keys[None, :, None]
    vals = np.einsum('vc,ncs->nsv', W_vals, ctx_flat) + b_vals[None, None, :]
    qk = np.einsum('bk,bks->bs', q, keys[batch_idx]) / math.sqrt(dk)
    qk = qk - qk.max(axis=-1, keepdims=True)
    e = np.exp(qk)
    coeff = e / e.sum(axis=-1, keepdims=True)
    ctx_out = np.einsum('bs,bsv->bv', coeff, vals[batch_idx])
    attn = coeff.reshape(-1, t, h, w) * 255.0
    return ctx_out.astype(np.float32), attn.astype(np.float32)


def kernel(rois, context, batch_idx, W_proj, b_proj, W_keys, b_keys,
           W_vals, b_vals):
    rois = np.asarray(rois, np.float32)
    context = np.asarray(context, np.float32)
    batch_idx = np.asarray(batch_idx)
    W_proj = np.asarray(W_proj, np.float32)
    b_proj = np.asarray(b_proj, np.float32)
    W_keys = np.asarray(W_keys, np.float32)
    b_keys = np.asarray(b_keys, np.float32)
    W_vals = np.asarray(W_vals, np.float32)
    b_vals = np.asarray(b_vals, np.float32)

    nb = rois.shape[0]
    if (context.shape != (N, C, T, H, W) or rois.shape[1] != DIN
            or W_proj.shape != (DK, DIN) or W_keys.shape != (DK, C)
            or W_vals.shape != (DV, C)):
        return _numpy_fallback(rois, context, batch_idx, W_proj, b_proj,
                               W_keys, b_keys, W_vals, b_vals)

    from concourse.bass_utils import run_bass_kernel_spmd

    nc = _get_nc()
    bf = ml_dtypes.bfloat16

    ctx_flat = context.reshape(N, C, S)
    # [N, NT, 128, CCH, TS]: partition p owns channels {cc*128+p}, 4KB contig
    ctx_bf = np.ascontiguousarray(
        ctx_flat.reshape(N, CCH, 128, NT, TS).transpose(0, 3, 2, 1, 4)
    ).astype(bf)

    wproj_h = np.ascontiguousarray(
        W_proj.T.reshape(ICH, 128, DK).transpose(1, 0, 2)).astype(bf)
    wkeys_h = np.ascontiguousarray(W_keys.reshape(DK, CCH, 128)).astype(bf)
    wvals_h = np.ascontiguousarray(
        W_vals.T.reshape(CCH, 128, DV).transpose(1, 0, 2)).astype(bf)
    bproj_h = np.ascontiguousarray(b_proj.reshape(DK, 1))
    bkeys_h = np.ascontiguousarray(b_keys.reshape(DK, 1)).astype(bf)
    bvals_h = np.ascontiguousarray(np.broadcast_to(b_vals, (128, CCH, DV)))
    ident_h = np.eye(128, dtype=np.float32).astype(bf)

    idxs = [np.flatnonzero(batch_idx == n) for n in range(N)]
    rounds = max(1, max((len(ix) + NBP - 1) // NBP for ix in idxs))

    ctx_out_full = np.zeros((nb, DV), np.float32)
    attn_full = np.zeros((nb, S), np.float32)

    for r in range(rounds):
        in_maps = []
        sels = []
        for n in range(N):
            ix = idxs[n][r * NBP:(r + 1) * NBP]
            sels.append(ix)
            rp = np.zeros((NBP, DIN), np.float32)
            if len(ix):
                rp[:len(ix)] = rois[ix]
            rois_h = np.ascontiguousarray(
                rp.T.reshape(ICH, 128, NBP).transpose(1, 0, 2)).astype(bf)
            in_maps.append({
                "ctx": ctx_bf[n], "roisT": rois_h, "wprojT": wproj_h,
                "wkeysK": wkeys_h, "wvalsT": wvals_h, "bproj": bproj_h,
                "bkeys": bkeys_h, "bvalsb": bvals_h, "ident": ident_h,
            })
        res = run_bass_kernel_spmd(nc, in_maps, list(range(N_CORES)))
        for n in range(N):
            ix = sels[n]
            if len(ix):
                k = len(ix)
                ctx_out_full[ix] = res.results[n]["ctx_out"][:k]
                e = res.results[n]["expout"][:k].astype(np.float32)
                scale = 255.0 / res.results[n]["sums"][:k]
                attn_full[ix] = e * scale
    return ctx_out_full, attn_full.reshape(nb, T, H, W)


# revision 4
# speedup vs baseline: 1.7225x; 1.1943x over previous
"""Trainium2 Bass kernel for the ContextModule sparse-attention problem.

Contract: kernel(**inputs) takes FULL unsharded numpy inputs and returns the
FULL outputs (ctx_out [512,128] f32, attn [512,16,32,32] f32).

Sharding: data-parallel over the clip dim N=8 — one clip per NeuronCore.
Each core gets its clip's context plus (up to 128, zero-padded) ROIs that
belong to that clip; projection weights are replicated. Host does the
gather/pad/layout and the final attn normalization; device does everything
contraction-shaped.

Device-side math per core (b = padded boxes, s = 16384 positions):
  qT[k,b]    = W_proj @ roisT + b_proj            (16 chunk matmuls)
  qWT[c,b]   = W_keys-chunk^T @ qT, scaled 1/sqrt(dk)   (fold keys away)
  WVQ        = [W_vals^T-chunk | qWT-chunk] per channel chunk (256 cols)
  per 512-col tile of ctx (s-subchunks j of 128):
    M[s,{v,b}] = sum_cc ctx-chunk^T @ WVQ_cc     (vals AND qk^T in one pass)
    e[s,b]     = exp(M[:,128:256])  (bf16; b_keys bias is softmax-invariant)
    vals[s,v'] = M[:,0:128] + b_vals, col 128.. = 1.0 (ones for row sums)
    acc[b,v']+= e-chunk^T @ vals-chunk    (col 128 accumulates sum_s e[b])
  ctx_out = acc[:,0:128] / acc[:,128] ; e rows + sums DMA'd out, host
  computes attn = e * 255/sum (transposing [s,b] -> [b,s]).
"""

import math
import sys

for _p in ("/opt/trn_rl_repo", "/root/.axon_site/_ro/trn_rl_repo"):
    if _p not in sys.path:
        sys.path.insert(0, _p)

import numpy as np
import ml_dtypes

# Problem shapes (hardcoded per contract)
N, C, T, H, W = 8, 512, 16, 32, 32
S = T * H * W            # 16384 spatial positions per clip
DIN, DK, DV = 2048, 128, 128
NBP = 128                # padded boxes per core per round
TS = 512                 # spatial tile
NT = S // TS             # 32 tiles
CCH = C // 128           # 4 contraction chunks over channels
ICH = DIN // 128         # 16 contraction chunks over roi features
VW = 132                 # vals row width: 128 vals + ones col + pad
N_CORES = 8

_STATE = {}


def _build():
    import concourse.bacc as bacc
    import concourse.mybir as mybir
    import concourse.tile as tile
    import concourse.bass as bass
    from contextlib import ExitStack

    f32 = mybir.dt.float32
    bf16 = mybir.dt.bfloat16
    AF = mybir.ActivationFunctionType

    nc = bacc.Bacc("TRN2", target_bir_lowering=False, debug=False,
                   num_devices=N_CORES)

    # ctx layout: [NT, 128(part), CCH, TS] — 4KB/partition contiguous per tile
    ctx_d = nc.dram_tensor("ctx", [NT, 128, CCH, TS], bf16, kind="ExternalInput").ap()
    rois_d = nc.dram_tensor("roisT", [128, ICH, NBP], bf16, kind="ExternalInput").ap()
    wproj_d = nc.dram_tensor("wprojT", [128, ICH, DK], bf16, kind="ExternalInput").ap()
    wkeys_d = nc.dram_tensor("wkeysK", [128, CCH, 128], bf16, kind="ExternalInput").ap()
    wvals_d = nc.dram_tensor("wvalsT", [128, CCH, DV], bf16, kind="ExternalInput").ap()
    bproj_d = nc.dram_tensor("bproj", [128, 1], f32, kind="ExternalInput").ap()
    bvals_d = nc.dram_tensor("bvalsb", [128, CCH, DV], f32, kind="ExternalInput").ap()
    ctxout_d = nc.dram_tensor("ctx_out", [NBP, DV], f32, kind="ExternalOutput").ap()
    expout_d = nc.dram_tensor("expout", [NT, 128, TS], bf16, kind="ExternalOutput").ap()
    sums_d = nc.dram_tensor("sums", [NBP, 1], f32, kind="ExternalOutput").ap()

    inv_sqrt_dk = 1.0 / math.sqrt(DK)

    with tile.TileContext(nc) as tc, ExitStack() as ctx:
        const = ctx.enter_context(tc.tile_pool(name="const", bufs=1))
        big = ctx.enter_context(tc.tile_pool(name="big", bufs=1))
        ctxp = ctx.enter_context(tc.tile_pool(name="ctxp", bufs=6))
        outp = ctx.enter_context(tc.tile_pool(name="outp", bufs=2))
        ps_m = ctx.enter_context(tc.tile_pool(name="ps_m", bufs=2, space="PSUM"))
        ps_t = ctx.enter_context(tc.tile_pool(name="ps_t", bufs=2, space="PSUM"))
        ps_acc = ctx.enter_context(tc.tile_pool(name="ps_acc", bufs=1, space="PSUM"))

        # ---- constants / persistent buffers ----
        rois_t = const.tile([128, ICH, NBP], bf16)
        wproj_t = const.tile([128, ICH, DK], bf16)
        wkeys_t = const.tile([128, CCH, 128], bf16)
        wvq_t = const.tile([128, CCH, 256], bf16)   # [vals | qWT] moving operand
        bproj_t = const.tile([128, 1], f32)
        bvals_t = const.tile([128, CCH, DV], f32)
        qT_t = const.tile([128, NBP], bf16)
        total_t = const.tile([128, 1], f32)
        inv_t = const.tile([128, 1], f32)

        exp_b = big.tile([128, NT, CCH, 128], bf16)   # e^T chunks [s, b]
        vals_sv = big.tile([128, NT, CCH, VW], bf16)  # vals + ones col

        # constant DMAs: split the big ones across queues for parallelism
        for h in range(4):
            nc.sync.dma_start(rois_t[:, h * 4:(h + 1) * 4, :],
                              rois_d[:, h * 4:(h + 1) * 4, :])
            nc.sync.dma_start(wproj_t[:, h * 4:(h + 1) * 4, :],
                              wproj_d[:, h * 4:(h + 1) * 4, :])
        nc.sync.dma_start(wkeys_t[:], wkeys_d[:])
        nc.sync.dma_start(wvq_t[:, :, 0:128], wvals_d[:])
        nc.sync.dma_start(bproj_t[:], bproj_d[:])
        nc.sync.dma_start(bvals_t[:], bvals_d[:])

        # ones (and pad) columns for the row-sum trick
        nc.vector.memset(vals_sv[:, :, :, 128:VW], 1.0)

        # ---- qT[k, b] = W_proj @ roisT + b_proj ----
        q_ps = ps_t.tile([128, NBP], f32, tag="tp")
        for ic in range(ICH):
            nc.tensor.matmul(q_ps[:], wproj_t[:, ic, :], rois_t[:, ic, :],
                             start=(ic == 0), stop=(ic == ICH - 1))
        nc.scalar.activation(qT_t[:], q_ps[:], AF.Identity, bias=bproj_t[:])

        # ---- qWT[c, b] chunks, pre-scaled by 1/sqrt(dk) ----
        qw_ps = ps_t.tile([128, CCH, NBP], f32, tag="tp")
        for cc in range(CCH):
            nc.tensor.matmul(qw_ps[:, cc, :], wkeys_t[:, cc, :], qT_t[:],
                             start=True, stop=True)
        nc.scalar.mul(wvq_t[:, :, 128:256], qw_ps[:], inv_sqrt_dk)

        # ---- main loop over spatial tiles ----
        acc_ps = ps_acc.tile([128, VW], f32)
        for ts in range(NT):
            ctile = ctxp.tile([128, CCH, TS], bf16)
            nc.sync.dma_start(ctile[:], ctx_d[ts])

            # fused vals + qk^T: M[s, {v,b}] per s-subchunk j
            mps = ps_m.tile([128, CCH, 256], f32)
            for j in range(CCH):
                for cc in range(CCH):
                    nc.tensor.matmul(mps[:, j, :],
                                     ctile[:, cc, bass.ts(j, 128)],
                                     wvq_t[:, cc, :],
                                     start=(cc == 0), stop=(cc == CCH - 1))

            nc.scalar.activation(exp_b[:, ts, :, :], mps[:, :, 128:256], AF.Exp)
            nc.vector.tensor_add(vals_sv[:, ts, :, 0:128], mps[:, :, 0:128],
                                 bvals_t[:])

            for j in range(CCH):
                g = ts * CCH + j
                nc.tensor.matmul(acc_ps[:], exp_b[:, ts, j, :],
                                 vals_sv[:, ts, j, :],
                                 start=(g == 0), stop=(g == NT * CCH - 1))

            # stream e^T rows out (host transposes + normalizes)
            nc.gpsimd.dma_start(expout_d[ts], exp_b[:, ts, :, :])

        # ---- outputs ----
        nc.vector.tensor_copy(total_t[:], acc_ps[:, 128:129])
        nc.vector.reciprocal(inv_t[:], total_t[:])
        nc.sync.dma_start(sums_d[:], total_t[:])

        ctxout_sb = outp.tile([128, DV], f32)
        nc.vector.tensor_scalar_mul(ctxout_sb[:], acc_ps[:, 0:128], inv_t[:])
        nc.sync.dma_start(ctxout_d[:], ctxout_sb[:])

    nc.compile()
    return nc


def _get_nc():
    if "nc" not in _STATE:
        _STATE["nc"] = _build()
    return _STATE["nc"]


def _numpy_fallback(rois, context, batch_idx, W_proj, b_proj, W_keys, b_keys,
                    W_vals, b_vals):
    n, c, t, h, w = context.shape
    s = t * h * w
    dk = W_keys.shape[0]
    q = rois @ W_proj.T + b_proj
    ctx_flat = context.reshape(n, c, s)
    keys = np.einsum('kc,ncs->nks', W_keys, ctx_flat) + b_keys[None, :, None]
    vals = np.einsum('vc,ncs->nsv', W_vals, ctx_flat) + b_vals[None, None, :]
    qk = np.einsum('bk,bks->bs', q, keys[batch_idx]) / math.sqrt(dk)
    qk = qk - qk.max(axis=-1, keepdims=True)
    e = np.exp(qk)
    coeff = e / e.sum(axis=-1, keepdims=True)
    ctx_out = np.einsum('bs,bsv->bv', coeff, vals[batch_idx])
    attn = coeff.reshape(-1, t, h, w) * 255.0
    return ctx_out.astype(np.float32), attn.astype(np.float32)


def kernel(rois, context, batch_idx, W_proj, b_proj, W_keys, b_keys,
           W_vals, b_vals):
    rois = np.asarray(rois, np.float32)
    context = np.asarray(context, np.float32)
    batch_idx = np.asarray(batch_idx)
    W_proj = np.asarray(W_proj, np.float32)
    b_proj = np.asarray(b_proj, np.float32)
    W_keys = np.asarray(W_keys, np.float32)
    b_keys = np.asarray(b_keys, np.float32)
    W_vals = np.asarray(W_vals, np.float32)
    b_vals = np.asarray(b_vals, np.float32)

    nb = rois.shape[0]
    # b_keys shifts every logit of a row equally -> cancels in softmax; the
    # device kernel drops it. Guard against pathological magnitudes anyway.
    if (context.shape != (N, C, T, H, W) or rois.shape[1] != DIN
            or W_proj.shape != (DK, DIN) or W_keys.shape != (DK, C)
            or W_vals.shape != (DV, C) or np.abs(b_keys).max() > 1.0):
        return _numpy_fallback(rois, context, batch_idx, W_proj, b_proj,
                               W_keys, b_keys, W_vals, b_vals)

    from concourse.bass_utils import run_bass_kernel_spmd

    nc = _get_nc()
    bf = ml_dtypes.bfloat16

    ctx_flat = context.reshape(N, C, S)
    # [N, NT, 128, CCH, TS]: partition p owns channels {cc*128+p}, 4KB contig
    ctx_bf = np.ascontiguousarray(
        ctx_flat.reshape(N, CCH, 128, NT, TS).transpose(0, 3, 2, 1, 4)
    ).astype(bf)

    wproj_h = np.ascontiguousarray(
        W_proj.T.reshape(ICH, 128, DK).transpose(1, 0, 2)).astype(bf)
    wkeys_h = np.ascontiguousarray(W_keys.reshape(DK, CCH, 128)).astype(bf)
    wvals_h = np.ascontiguousarray(
        W_vals.T.reshape(CCH, 128, DV).transpose(1, 0, 2)).astype(bf)
    bproj_h = np.ascontiguousarray(b_proj.reshape(DK, 1))
    bvals_h = np.ascontiguousarray(np.broadcast_to(b_vals, (128, CCH, DV)))

    idxs = [np.flatnonzero(batch_idx == n) for n in range(N)]
    rounds = max(1, max((len(ix) + NBP - 1) // NBP for ix in idxs))

    ctx_out_full = np.zeros((nb, DV), np.float32)
    attn_full = np.zeros((nb, S), np.float32)

    for r in range(rounds):
        in_maps = []
        sels = []
        for n in range(N):
            ix = idxs[n][r * NBP:(r + 1) * NBP]
            sels.append(ix)
            rp = np.zeros((NBP, DIN), np.float32)
            if len(ix):
                rp[:len(ix)] = rois[ix]
            rois_h = np.ascontiguousarray(
                rp.T.reshape(ICH, 128, NBP).transpose(1, 0, 2)).astype(bf)
            in_maps.append({
                "ctx": ctx_bf[n], "roisT": rois_h, "wprojT": wproj_h,
                "wkeysK": wkeys_h, "wvalsT": wvals_h, "bproj": bproj_h,
                "bvalsb": bvals_h,
            })
        res = run_bass_kernel_spmd(nc, in_maps, list(range(N_CORES)))
        for n in range(N):
            ix = sels[n]
            if len(ix):
                k = len(ix)
                ctx_out_full[ix] = res.results[n]["ctx_out"][:k]
                # expout: [NT, s_in 128, (j CCH, b 128)] -> [b, s]
                e = np.asarray(res.results[n]["expout"]).reshape(NT, 128, CCH, 128)
                e = e.transpose(3, 0, 2, 1).reshape(NBP, S)[:k].astype(np.float32)
                scale = 255.0 / res.results[n]["sums"][:k]
                attn_full[ix] = e * scale
    return ctx_out_full, attn_full.reshape(nb, T, H, W)


# revision 6
# speedup vs baseline: 1.7353x; 1.0074x over previous
"""Trainium2 Bass kernel for the ContextModule sparse-attention problem.

Contract: kernel(**inputs) takes FULL unsharded numpy inputs and returns the
FULL outputs (ctx_out [512,128] f32, attn [512,16,32,32] f32).

Sharding: data-parallel over the clip dim N=8 — one clip per NeuronCore.
Each core gets its clip's context plus (up to 128, zero-padded) ROIs that
belong to that clip; projection weights are replicated. Host does the
gather/pad/layout and the final attn normalization; device does everything
contraction-shaped.

Device-side math per core (b = padded boxes, s = 16384 positions):
  qT[k,b]    = W_proj @ roisT + b_proj            (16 chunk matmuls)
  qWT[c,b]   = W_keys-chunk^T @ qT, scaled 1/sqrt(dk)   (fold keys away)
  WVQ        = [W_vals^T-chunk | qWT-chunk] per channel chunk (256 cols)
  per 512-col tile of ctx (s-subchunks j of 128):
    M[s,{v,b}] = sum_cc ctx-chunk^T @ WVQ_cc     (vals AND qk^T in one pass)
    e[s,b]     = exp(M[:,128:256])  (bf16; b_keys bias is softmax-invariant)
    vals[s,v'] = M[:,0:128] + b_vals, col 128.. = 1.0 (ones for row sums)
    acc[b,v']+= e-chunk^T @ vals-chunk    (col 128 accumulates sum_s e[b])
  ctx_out = acc[:,0:128] / acc[:,128] ; e rows + sums DMA'd out, host
  computes attn = e * 255/sum (transposing [s,b] -> [b,s]).
"""

import math
import sys

for _p in ("/opt/trn_rl_repo", "/root/.axon_site/_ro/trn_rl_repo"):
    if _p not in sys.path:
        sys.path.insert(0, _p)

import numpy as np
import ml_dtypes

# Problem shapes (hardcoded per contract)
N, C, T, H, W = 8, 512, 16, 32, 32
S = T * H * W            # 16384 spatial positions per clip
DIN, DK, DV = 2048, 128, 128
NBP = 128                # padded boxes per core per round
TS = 512                 # spatial tile
NT = S // TS             # 32 tiles
CCH = C // 128           # 4 contraction chunks over channels
ICH = DIN // 128         # 16 contraction chunks over roi features
VW = 132                 # vals row width: 128 vals + ones col + pad
N_CORES = 8

_STATE = {}


def _build():
    import concourse.bacc as bacc
    import concourse.mybir as mybir
    import concourse.tile as tile
    import concourse.bass as bass
    from contextlib import ExitStack

    f32 = mybir.dt.float32
    bf16 = mybir.dt.bfloat16
    AF = mybir.ActivationFunctionType

    nc = bacc.Bacc("TRN2", target_bir_lowering=False, debug=False,
                   num_devices=N_CORES, num_swdge_queues=4)

    # ctx layout: [NT, 128(part), CCH, TS] — 4KB/partition contiguous per tile
    ctx_d = nc.dram_tensor("ctx", [NT, 128, CCH, TS], bf16, kind="ExternalInput").ap()
    rois_d = nc.dram_tensor("roisT", [128, ICH, NBP], bf16, kind="ExternalInput").ap()
    wproj_d = nc.dram_tensor("wprojT", [128, ICH, DK], bf16, kind="ExternalInput").ap()
    wkeys_d = nc.dram_tensor("wkeysK", [128, CCH, 128], bf16, kind="ExternalInput").ap()
    wvals_d = nc.dram_tensor("wvalsT", [128, CCH, DV], bf16, kind="ExternalInput").ap()
    bproj_d = nc.dram_tensor("bproj", [128, 1], f32, kind="ExternalInput").ap()
    bvals_d = nc.dram_tensor("bvalsb", [128, CCH, DV], f32, kind="ExternalInput").ap()
    ctxout_d = nc.dram_tensor("ctx_out", [NBP, DV], f32, kind="ExternalOutput").ap()
    expout_d = nc.dram_tensor("expout", [NT, 128, TS], bf16, kind="ExternalOutput").ap()
    sums_d = nc.dram_tensor("sums", [NBP, 1], f32, kind="ExternalOutput").ap()

    inv_sqrt_dk = 1.0 / math.sqrt(DK)

    with tile.TileContext(nc) as tc, ExitStack() as ctx:
        const = ctx.enter_context(tc.tile_pool(name="const", bufs=1))
        big = ctx.enter_context(tc.tile_pool(name="big", bufs=1))
        ctxp = ctx.enter_context(tc.tile_pool(name="ctxp", bufs=8))
        outp = ctx.enter_context(tc.tile_pool(name="outp", bufs=2))
        ps_m = ctx.enter_context(tc.tile_pool(name="ps_m", bufs=2, space="PSUM"))
        ps_t = ctx.enter_context(tc.tile_pool(name="ps_t", bufs=2, space="PSUM"))
        ps_acc = ctx.enter_context(tc.tile_pool(name="ps_acc", bufs=1, space="PSUM"))
        ps_w = ctx.enter_context(tc.tile_pool(name="ps_w", bufs=1, space="PSUM"))

        # ---- constants / persistent buffers ----
        rois_t = const.tile([128, ICH, NBP], bf16)
        wproj_t = const.tile([128, ICH, DK], bf16)
        wkeys_t = const.tile([128, CCH, 128], bf16)
        wvq_t = const.tile([128, CCH, 256], bf16)   # [vals | qWT] moving operand
        bproj_t = const.tile([128, 1], f32)
        bvals_t = const.tile([128, CCH, DV], f32)
        qT_t = const.tile([128, NBP], bf16)
        total_t = const.tile([128, 1], f32)
        inv_t = const.tile([128, 1], f32)

        exp_b = big.tile([128, NT, CCH, 128], bf16)   # e^T chunks [s, b]
        vals_sv = big.tile([128, NT, CCH, VW], bf16)  # vals + ones col

        # PE warm-up: dependency-free matmuls so HAM un-throttles to 2.4GHz
        # while the constant DMAs land; they overlap the ramp, not real work.
        warm_t = const.tile([128, 128], bf16)
        nc.vector.memset(warm_t[:], 0.0)
        warm_ps = ps_w.tile([128, 64], f32)
        for _ in range(48):
            nc.tensor.matmul(warm_ps[:], warm_t[:], warm_t[:, 0:64],
                             start=True, stop=True)

        # constant DMAs: split the big ones across queues for parallelism
        for h in range(4):
            nc.sync.dma_start(rois_t[:, h * 4:(h + 1) * 4, :],
                              rois_d[:, h * 4:(h + 1) * 4, :])
            nc.sync.dma_start(wproj_t[:, h * 4:(h + 1) * 4, :],
                              wproj_d[:, h * 4:(h + 1) * 4, :])
        nc.sync.dma_start(wkeys_t[:], wkeys_d[:])
        nc.sync.dma_start(wvq_t[:, :, 0:128], wvals_d[:])
        nc.sync.dma_start(bproj_t[:], bproj_d[:])
        nc.sync.dma_start(bvals_t[:], bvals_d[:])

        # ones (and pad) columns for the row-sum trick
        nc.vector.memset(vals_sv[:, :, :, 128:VW], 1.0)

        # ---- qT[k, b] = W_proj @ roisT + b_proj ----
        q_ps = ps_t.tile([128, NBP], f32, tag="tp")
        for ic in range(ICH):
            nc.tensor.matmul(q_ps[:], wproj_t[:, ic, :], rois_t[:, ic, :],
                             start=(ic == 0), stop=(ic == ICH - 1))
        nc.scalar.activation(qT_t[:], q_ps[:], AF.Identity, bias=bproj_t[:])

        # ---- qWT[c, b] chunks, pre-scaled by 1/sqrt(dk) ----
        qw_ps = ps_t.tile([128, CCH, NBP], f32, tag="tp")
        for cc in range(CCH):
            nc.tensor.matmul(qw_ps[:, cc, :], wkeys_t[:, cc, :], qT_t[:],
                             start=True, stop=True)
        nc.scalar.mul(wvq_t[:, :, 128:256], qw_ps[:], inv_sqrt_dk)

        # ---- main loop over spatial tiles ----
        acc_ps = ps_acc.tile([128, VW], f32)
        for ts in range(NT):
            ctile = ctxp.tile([128, CCH, TS], bf16)
            nc.sync.dma_start(ctile[:], ctx_d[ts])

            # fused vals + qk^T: M[s, {v,b}] per s-subchunk j
            mps = ps_m.tile([128, CCH, 256], f32)
            for j in range(CCH):
                for cc in range(CCH):
                    nc.tensor.matmul(mps[:, j, :],
                                     ctile[:, cc, bass.ts(j, 128)],
                                     wvq_t[:, cc, :],
                                     start=(cc == 0), stop=(cc == CCH - 1))

            nc.scalar.activation(exp_b[:, ts, :, :], mps[:, :, 128:256], AF.Exp)
            nc.vector.tensor_add(vals_sv[:, ts, :, 0:128], mps[:, :, 0:128],
                                 bvals_t[:])

            for j in range(CCH):
                g = ts * CCH + j
                nc.tensor.matmul(acc_ps[:], exp_b[:, ts, j, :],
                                 vals_sv[:, ts, j, :],
                                 start=(g == 0), stop=(g == NT * CCH - 1))

            # stream e^T rows out (host transposes + normalizes)
            if ts % 2 == 0:
                nc.gpsimd.dma_start(expout_d[ts], exp_b[:, ts, :, :])
            else:
                nc.sync.dma_start(expout_d[ts], exp_b[:, ts, :, :])

        # ---- outputs ----
        nc.vector.tensor_copy(total_t[:], acc_ps[:, 128:129])
        nc.vector.reciprocal(inv_t[:], total_t[:])
        nc.sync.dma_start(sums_d[:], total_t[:])

        ctxout_sb = outp.tile([128, DV], f32)
        nc.vector.tensor_scalar_mul(ctxout_sb[:], acc_ps[:, 0:128], inv_t[:])
        nc.sync.dma_start(ctxout_d[:], ctxout_sb[:])

    nc.compile()
    return nc


def _get_nc():
    if "nc" not in _STATE:
        _STATE["nc"] = _build()
    return _STATE["nc"]


def _numpy_fallback(rois, context, batch_idx, W_proj, b_proj, W_keys, b_keys,
                    W_vals, b_vals):
    n, c, t, h, w = context.shape
    s = t * h * w
    dk = W_keys.shape[0]
    q = rois @ W_proj.T + b_proj
    ctx_flat = context.reshape(n, c, s)
    keys = np.einsum('kc,ncs->nks', W_keys, ctx_flat) + b_keys[None, :, None]
    vals = np.einsum('vc,ncs->nsv', W_vals, ctx_flat) + b_vals[None, None, :]
    qk = np.einsum('bk,bks->bs', q, keys[batch_idx]) / math.sqrt(dk)
    qk = qk - qk.max(axis=-1, keepdims=True)
    e = np.exp(qk)
    coeff = e / e.sum(axis=-1, keepdims=True)
    ctx_out = np.einsum('bs,bsv->bv', coeff, vals[batch_idx])
    attn = coeff.reshape(-1, t, h, w) * 255.0
    return ctx_out.astype(np.float32), attn.astype(np.float32)


def kernel(rois, context, batch_idx, W_proj, b_proj, W_keys, b_keys,
           W_vals, b_vals):
    rois = np.asarray(rois, np.float32)
    context = np.asarray(context, np.float32)
    batch_idx = np.asarray(batch_idx)
    W_proj = np.asarray(W_proj, np.float32)
    b_proj = np.asarray(b_proj, np.float32)
    W_keys = np.asarray(W_keys, np.float32)
    b_keys = np.asarray(b_keys, np.float32)
    W_vals = np.asarray(W_vals, np.float32)
    b_vals = np.asarray(b_vals, np.float32)

    nb = rois.shape[0]
    # b_keys shifts every logit of a row equally -> cancels in softmax; the
    # device kernel drops it. Guard against pathological magnitudes anyway.
    if (context.shape != (N, C, T, H, W) or rois.shape[1] != DIN
            or W_proj.shape != (DK, DIN) or W_keys.shape != (DK, C)
            or W_vals.shape != (DV, C) or np.abs(b_keys).max() > 1.0):
        return _numpy_fallback(rois, context, batch_idx, W_proj, b_proj,
                               W_keys, b_keys, W_vals, b_vals)

    from concourse.bass_utils import run_bass_kernel_spmd

    nc = _get_nc()
    bf = ml_dtypes.bfloat16

    ctx_flat = context.reshape(N, C, S)
    # [N, NT, 128, CCH, TS]: partition p owns channels {cc*128+p}, 4KB contig
    ctx_bf = np.ascontiguousarray(
        ctx_flat.reshape(N, CCH, 128, NT, TS).transpose(0, 3, 2, 1, 4)
    ).astype(bf)

    wproj_h = np.ascontiguousarray(
        W_proj.T.reshape(ICH, 128, DK).transpose(1, 0, 2)).astype(bf)
    wkeys_h = np.ascontiguousarray(W_keys.reshape(DK, CCH, 128)).astype(bf)
    wvals_h = np.ascontiguousarray(
        W_vals.T.reshape(CCH, 128, DV).transpose(1, 0, 2)).astype(bf)
    bproj_h = np.ascontiguousarray(b_proj.reshape(DK, 1))
    bvals_h = np.ascontiguousarray(np.broadcast_to(b_vals, (128, CCH, DV)))

    idxs = [np.flatnonzero(batch_idx == n) for n in range(N)]
    rounds = max(1, max((len(ix) + NBP - 1) // NBP for ix in idxs))

    ctx_out_full = np.zeros((nb, DV), np.float32)
    attn_full = np.zeros((nb, S), np.float32)

    for r in range(rounds):
        in_maps = []
        sels = []
        for n in range(N):
            ix = idxs[n][r * NBP:(r + 1) * NBP]
            sels.append(ix)
            rp = np.zeros((NBP, DIN), np.float32)
            if len(ix):
                rp[:len(ix)] = rois[ix]
            rois_h = np.ascontiguousarray(
                rp.T.reshape(ICH, 128, NBP).transpose(1, 0, 2)).astype(bf)
            in_maps.append({
                "ctx": ctx_bf[n], "roisT": rois_h, "wprojT": wproj_h,
                "wkeysK": wkeys_h, "wvalsT": wvals_h, "bproj": bproj_h,
                "bvalsb": bvals_h,
            })
        res = run_bass_kernel_spmd(nc, in_maps, list(range(N_CORES)))
        for n in range(N):
            ix = sels[n]
            if len(ix):
                k = len(ix)
                ctx_out_full[ix] = res.results[n]["ctx_out"][:k]
                # expout: [NT, s_in 128, (j CCH, b 128)] -> [b, s]
                e = np.asarray(res.results[n]["expout"]).reshape(NT, 128, CCH, 128)
                e = e.transpose(3, 0, 2, 1).reshape(NBP, S)[:k].astype(np.float32)
                scale = 255.0 / res.results[n]["sums"][:k]
                attn_full[ix] = e * scale
    return ctx_out_full, attn_full.reshape(nb, T, H, W)
